# revision 3
# baseline (speedup 1.0000x reference)
"""Trainium2 Bass kernel for nn_BHS_SAGE (GNN message passing + dueling head).

Node-chunk sharding: core c owns nodes [128c, 128(c+1)) of ALL 128 graphs
(instead of 16 whole graphs).  The SAGE stages (pool-MLP, edge max-agg,
self+neigh matmul) see the same per-core work either way, but the dueling
head only needs this core's 128-node slice of W_adv/W_v1 (2.4 MB instead of
the full 19.9 MB replicated), and its matmuls run at M=128 (full PE rows).
Per-graph head partials [128 g, 76] are summed across cores with a 39 KB
ReduceScatter; each core then computes the dueling tail for its own 16
output graphs.

Per-core pipeline, 8 slabs (each slab = 16 dst nodes x 32 graph-groups):
  A. z = W_pool_blockdiag @ xe-slab (4-graph packed, FD-1024 PSUM blocks)
  B. agg-max over 16 edge slots, drain alternating:
       even blocks: DVE reduce_max from PSUM, relu+bias applied after (DVE)
       odd  blocks: ACT relu+bias PSUM->SBUF bf16, then 4x DVE reduce_max
  D. h = relu(W_self x + W_neigh agg + b) per graph quadrant (ACT drain)
  E. head psum[128 g, 76] += ht[:, j].T @ whead[:, j]  (16 matmuls/slab,
     accumulated across all 128 j; hidden under the slab drains)
  F. ReduceScatter(add) -> [16 g, 76]; dueling tail; out [16, 12]
"""

import numpy as np

B, N, F, H, DEG = 128, 1024, 32, 128, 16
NCORES = 8
NC_ = N // NCORES         # 128 nodes per core chunk
BL = B // NCORES          # 16 output graphs per core (tail/out only)
GRP = B // 4              # 32 groups of 4 graphs packed per 128 partitions
NA = 12                   # adv outputs (3 branches x 4 actions)
NV = 64                   # val hidden
NH = NA + NV              # 76 combined head outputs
NSLAB = 8                 # j-slabs per core
JS = NC_ // NSLAB         # 16 dst nodes per slab
SLABC = JS * GRP * DEG    # 8192 xe cols per slab
BLK = 1024                # PSUM drain block (2 banks)

_CACHE = {}
LAST_RESULTS = None


def _build_program():
    import concourse.bass as bass
    import concourse.bacc as bacc
    import concourse.mybir as mybir
    import concourse.tile as tile

    f32 = mybir.dt.float32
    bf16 = mybir.dt.bfloat16
    Relu = mybir.ActivationFunctionType.Relu
    Alu = mybir.AluOpType

    nc = bacc.Bacc("TRN2", target_bir_lowering=False, debug=False,
                   num_devices=NCORES)

    # ---- kernel I/O ----
    xt_d = nc.declare_dram_parameter("xt", [128, NC_ * GRP], bf16, isOutput=False)
    xe_d = nc.declare_dram_parameter("xe", [128, NSLAB * SLABC], bf16, isOutput=False)
    wpool_d = nc.declare_dram_parameter("wpool_bd", [128, 128], bf16, isOutput=False)
    bpool_d = nc.declare_dram_parameter("bpool", [128, 1], f32, isOutput=False)
    wself_d = nc.declare_dram_parameter("wself_bd", [128, 4 * H], bf16, isOutput=False)
    wneigh_d = nc.declare_dram_parameter("wneigh_bd", [128, 4 * H], bf16, isOutput=False)
    bsage_d = nc.declare_dram_parameter("bsage", [128, 1], f32, isOutput=False)
    identf_d = nc.declare_dram_parameter("identf", [128, 128], f32, isOutput=False)
    whead_d = nc.declare_dram_parameter("whead", [128, NC_ * NH], bf16, isOutput=False)
    badv_d = nc.declare_dram_parameter("badv", [BL, NA], f32, isOutput=False)
    bv1_d = nc.declare_dram_parameter("bv1", [BL, NV], f32, isOutput=False)
    wv2_d = nc.declare_dram_parameter("wv2", [NV, NV], f32, isOutput=False)
    bv2_d = nc.declare_dram_parameter("bv2", [NV, 1], f32, isOutput=False)
    wv3_d = nc.declare_dram_parameter("wv3", [NV, 1], f32, isOutput=False)
    bv3_d = nc.declare_dram_parameter("bv3r", [BL, 1], f32, isOutput=False)
    out_d = nc.declare_dram_parameter("out", [BL, NA], f32, isOutput=True)

    import os as _os
    _dbg = _os.environ.get("KDBG") == "1"
    if _dbg:
        dbg_aggT_d = nc.declare_dram_parameter("dbg_aggT", [128, NC_ * GRP], bf16, isOutput=True)
        dbg_ht_d = nc.declare_dram_parameter("dbg_ht", [128, NC_ * B], bf16, isOutput=True)

    with tile.TileContext(nc) as tc:
        with (
            tc.tile_pool(name="const", bufs=1) as cpool,
            tc.tile_pool(name="big", bufs=1) as bigpool,
        ):
            # ---- constants / persistent tiles ----
            identf = cpool.tile([128, 128], f32)
            nc.sync.dma_start(out=identf[:], in_=identf_d[:])
            xt = cpool.tile([128, NC_ * GRP], bf16)        # [(q,f), (j,grp)]
            nc.sync.dma_start(out=xt[:], in_=xt_d[:])
            wpool = cpool.tile([128, 128], bf16)
            nc.sync.dma_start(out=wpool[:], in_=wpool_d[:])
            bpool = cpool.tile([128, 1], f32)
            nc.sync.dma_start(out=bpool[:], in_=bpool_d[:])
            wself = cpool.tile([128, 4 * H], bf16)
            nc.sync.dma_start(out=wself[:], in_=wself_d[:])
            wneigh = cpool.tile([128, 4 * H], bf16)
            nc.sync.dma_start(out=wneigh[:], in_=wneigh_d[:])
            bsage = cpool.tile([128, 1], f32)
            nc.sync.dma_start(out=bsage[:], in_=bsage_d[:])
            whead = cpool.tile([128, NC_ * NH], bf16)      # [h, (j, o)]
            nc.sync.dma_start(out=whead[:], in_=whead_d[:])

            ht = bigpool.tile([128, NC_ * B], bf16)        # [h, j*128 + g]  4MB
            aggT = bigpool.tile([128, NC_ * GRP], bf16)    # [(q,f'), j*32+grp] 1MB

            # head psum allocated up-front: accumulates across all slabs
            hd_ps_ctx = tc.tile_pool(name="hd_ps", bufs=1, space="PSUM")
            hd_ps = hd_ps_ctx.__enter__()
            pshead = hd_ps.tile([128, NH], f32)

            with (
                tc.tile_pool(name="xe_sb", bufs=3) as xe_pool,
                tc.tile_pool(name="z_ps", bufs=2, space="PSUM") as z_ps,
                tc.tile_pool(name="zr_sb", bufs=2) as zr_pool,
                tc.tile_pool(name="h_ps", bufs=2, space="PSUM") as h_ps,
            ):
                for s in range(NSLAB):
                    # ---- stage A+B: aggT slab = relu(max_d(W_pool@x[src]) + b) ----
                    # xe slab cols: (jj 16, grp 32, d 16); aggT cols: j*32+grp
                    xe = xe_pool.tile([128, SLABC], bf16, tag="xe")
                    nc.sync.dma_start(
                        out=xe[:], in_=xe_d[:, s * SLABC:(s + 1) * SLABC])
                    for blk in range(SLABC // BLK):   # 8 blocks of 1024 (64 nd, 16 d)
                        a0 = s * JS * GRP + blk * (BLK // DEG)  # aggT col offset
                        ps = z_ps.tile([128, BLK], f32, tag="zps")
                        for h2 in range(2):   # one matmul per PSUM bank
                            nc.tensor.matmul(
                                out=ps[:, h2 * 512:(h2 + 1) * 512],
                                lhsT=wpool[:],
                                rhs=xe[:, blk * BLK + h2 * 512:
                                        blk * BLK + (h2 + 1) * 512],
                                start=True, stop=True,
                            )
                        if blk % 2 == 0:
                            # DVE-direct: reduce from PSUM; relu+bias later
                            nc.vector.reduce_max(
                                out=aggT[:, a0:a0 + BLK // DEG],
                                in_=ps[:].rearrange("p (n d) -> p n d", d=DEG),
                                axis=mybir.AxisListType.X)
                        else:
                            # ACT-assisted: fused relu+bias drain, then 4x reduce
                            zr = zr_pool.tile([128, BLK], bf16, tag="zr")
                            nc.scalar.activation(
                                out=zr[:], in_=ps[:], func=Relu, bias=bpool[:])
                            nc.vector.reduce_max(
                                out=aggT[:, a0:a0 + BLK // DEG],
                                in_=zr[:].rearrange("p (n d) -> p n d", d=DEG),
                                axis=mybir.AxisListType.X)
                    # relu+bias for the DVE-direct strips (blocks 0,2,4,6)
                    av = aggT[:].rearrange("p (b n) -> p b n", n=BLK // DEG)
                    nc.vector.tensor_scalar(
                        out=av[:, 8 * s:8 * s + 8:2, :],
                        in0=av[:, 8 * s:8 * s + 8:2, :],
                        scalar1=bpool[:], scalar2=0.0,
                        op0=Alu.add, op1=Alu.max)

                    # ---- stage D: ht slab = relu(W_self x + W_neigh agg + b) ----
                    # rhs cols (jj 16, grp 32) contiguous; out ht[j*128+grp*4+q]
                    htv = ht[:].rearrange("p (j grp q) -> p j grp q", grp=GRP, q=4)
                    for q in range(4):
                        hp = h_ps.tile([128, JS * GRP], f32, tag="hps")
                        nc.tensor.matmul(
                            out=hp[:],
                            lhsT=wself[:, q * H:(q + 1) * H],
                            rhs=xt[:, s * JS * GRP:(s + 1) * JS * GRP],
                            start=True, stop=False)
                        nc.tensor.matmul(
                            out=hp[:],
                            lhsT=wneigh[:, q * H:(q + 1) * H],
                            rhs=aggT[:, s * JS * GRP:(s + 1) * JS * GRP],
                            start=False, stop=True)
                        nc.scalar.activation(
                            out=htv[:, s * JS:(s + 1) * JS, :, q],
                            in_=hp[:].rearrange("p (j grp) -> p j grp", grp=GRP),
                            func=Relu, bias=bsage[:])

                    # ---- stage E: head accumulation over this slab's nodes ----
                    for jj in range(JS):
                        j = s * JS + jj
                        nc.tensor.matmul(
                            out=pshead[:],
                            lhsT=ht[:, j * B:(j + 1) * B],
                            rhs=whead[:, j * NH:(j + 1) * NH],
                            start=(j == 0), stop=(j == NC_ - 1),
                        )

            if _dbg:
                nc.sync.dma_start(out=dbg_aggT_d[:], in_=aggT[:])
                nc.sync.dma_start(out=dbg_ht_d[:], in_=ht[:])

            # ---- stage F: cross-core reduce + dueling tail ----
            with (
                tc.tile_pool(name="tail", bufs=1) as tp,
                tc.tile_pool(name="dram", bufs=1, space="DRAM") as dram,
            ):
                ccin = dram.tile([128, NH], f32)
                ccout = dram.tile([BL, NH], f32)
                psf = tp.tile([128, NH], f32)
                nc.scalar.copy(out=psf[:], in_=pshead[:])
                nc.gpsimd.dma_start(out=ccin[:], in_=psf[:])
                nc.gpsimd.collective_compute(
                    "ReduceScatter",
                    mybir.AluOpType.add,
                    replica_groups=[list(range(NCORES))],
                    ins=[ccin[:].opt()],
                    outs=[ccout[:].opt()],
                )
                hsum = tp.tile([BL, NH], f32)
                nc.gpsimd.dma_start(out=hsum[:], in_=ccout[:])

                badv = tp.tile([BL, NA], f32)
                nc.sync.dma_start(out=badv[:], in_=badv_d[:])
                bv1 = tp.tile([BL, NV], f32)
                nc.sync.dma_start(out=bv1[:], in_=bv1_d[:])
                wv2 = tp.tile([NV, NV], f32)
                nc.sync.dma_start(out=wv2[:], in_=wv2_d[:])
                bv2 = tp.tile([NV, 1], f32)
                nc.sync.dma_start(out=bv2[:], in_=bv2_d[:])
                wv3 = tp.tile([NV, 1], f32)
                nc.sync.dma_start(out=wv3[:], in_=wv3_d[:])
                bv3 = tp.tile([BL, 1], f32)
                nc.sync.dma_start(out=bv3[:], in_=bv3_d[:])

                adv = tp.tile([BL, NA], f32)
                nc.vector.tensor_tensor(
                    out=adv[:], in0=hsum[:, 0:NA], in1=badv[:], op=Alu.add)
                nc.vector.tensor_scalar_max(adv[:], adv[:], 0.0)
                val1 = tp.tile([BL, NV], f32)
                nc.vector.tensor_tensor(
                    out=val1[:], in0=hsum[:, NA:NH], in1=bv1[:], op=Alu.add)
                nc.vector.tensor_scalar_max(val1[:], val1[:], 0.0)

                with tc.tile_pool(name="tl_ps", bufs=2, space="PSUM") as tl_ps:
                    # val1 [16, 64] -> val1T [64, 16]
                    pst = tl_ps.tile([NV, BL], f32, tag="a")
                    nc.tensor.transpose(
                        out=pst[:], in_=val1[:], identity=identf[0:BL, 0:BL])
                    val1T = tp.tile([NV, BL], f32)
                    nc.scalar.copy(out=val1T[:], in_=pst[:])
                    # val2T [64, 16] = relu(W_v2 @ val1 + b_v2)
                    ps2 = tl_ps.tile([NV, BL], f32, tag="b")
                    nc.tensor.matmul(
                        out=ps2[:], lhsT=wv2[:], rhs=val1T[:], start=True, stop=True)
                    val2T = tp.tile([NV, BL], f32)
                    nc.scalar.activation(
                        out=val2T[:], in_=ps2[:], func=Relu, bias=bv2[:])
                    # val3 [16, 1]
                    ps3 = tl_ps.tile([BL, 1], f32, tag="a")
                    nc.tensor.matmul(
                        out=ps3[:], lhsT=val2T[:], rhs=wv3[:], start=True, stop=True)
                    val3 = tp.tile([BL, 1], f32)
                    nc.vector.tensor_tensor(
                        out=val3[:], in0=ps3[:], in1=bv3[:], op=Alu.add)

                # out = val + adv - mean_j(adv)
                m = tp.tile([BL, 3], f32)
                nc.vector.reduce_sum(
                    out=m[:],
                    in_=adv[:].rearrange("p (a b) -> p a b", b=4),
                    axis=mybir.AxisListType.X)
                nc.vector.tensor_scalar_mul(m[:], m[:], 0.25)
                outt = tp.tile([BL, NA], f32)
                nc.vector.tensor_tensor(
                    out=outt[:], in0=adv[:],
                    in1=val3[:].to_broadcast([BL, NA]), op=Alu.add)
                nc.vector.tensor_tensor(
                    out=outt[:].rearrange("p (a b) -> p a b", b=4),
                    in0=outt[:].rearrange("p (a b) -> p a b", b=4),
                    in1=m[:].to_broadcast([BL, 3, 4]),
                    op=Alu.subtract)
                nc.sync.dma_start(out=out_d[:], in_=outt[:])
            hd_ps_ctx.__exit__(None, None, None)
    nc.compile()
    return nc


def _make_in_maps(inputs):
    import ml_dtypes
    bf = ml_dtypes.bfloat16

    x = np.asarray(inputs["x"], np.float32)
    src = np.asarray(inputs["src"], np.int64)
    W_pool = np.asarray(inputs["W_pool"], np.float32)
    b_pool = np.asarray(inputs["b_pool"], np.float32)
    W_self = np.asarray(inputs["W_self"], np.float32)
    W_neigh = np.asarray(inputs["W_neigh"], np.float32)
    b_sage = np.asarray(inputs["b_sage"], np.float32)
    W_adv = np.asarray(inputs["W_adv"], np.float32)
    b_adv = np.asarray(inputs["b_adv"], np.float32)
    W_v1 = np.asarray(inputs["W_v1"], np.float32)
    b_v1 = np.asarray(inputs["b_v1"], np.float32)
    W_v2 = np.asarray(inputs["W_v2"], np.float32)
    b_v2 = np.asarray(inputs["b_v2"], np.float32)
    W_v3 = np.asarray(inputs["W_v3"], np.float32)
    b_v3 = np.asarray(inputs["b_v3"], np.float32)

    # shared (replicated) tensors
    wpool_bd = np.kron(np.eye(4, dtype=np.float32), W_pool.T).astype(bf)  # [128,128]
    bpool = np.ascontiguousarray(np.tile(b_pool, 4)[:, None], np.float32)
    wself_bd = np.zeros((128, 4 * H), np.float32)
    wneigh_bd = np.zeros((128, 4 * H), np.float32)
    for q in range(4):
        wself_bd[q * 32:(q + 1) * 32, q * H:(q + 1) * H] = W_self.T
        wneigh_bd[q * 32:(q + 1) * 32, q * H:(q + 1) * H] = W_neigh.T
    bsage = np.ascontiguousarray(b_sage[:, None])
    W_cat = np.concatenate([W_adv, W_v1], axis=0)          # [76, 131072]
    W_cat = W_cat.reshape(NH, N, H)                        # [o, n, h]
    badv = np.ascontiguousarray(np.broadcast_to(b_adv[None, :], (BL, NA)))
    bv1 = np.ascontiguousarray(np.broadcast_to(b_v1[None, :], (BL, NV)))
    wv2 = np.ascontiguousarray(W_v2.T)
    bv2 = np.ascontiguousarray(b_v2[:, None])
    wv3 = np.ascontiguousarray(W_v3.T)
    bv3r = np.full((BL, 1), float(b_v3[0]), np.float32)
    ident = np.eye(128, dtype=np.float32)

    shared = {
        "wpool_bd": wpool_bd, "bpool": bpool,
        "wself_bd": wself_bd.astype(bf), "wneigh_bd": wneigh_bd.astype(bf),
        "bsage": bsage, "badv": badv,
        "bv1": bv1, "wv2": wv2, "bv2": bv2, "wv3": wv3, "bv3r": bv3r,
        "identf": ident,
    }

    xbf = x.astype(bf)                                     # [128, 1024, 32]
    sidx = (src.reshape(B, N, DEG)
            - (np.arange(B, dtype=np.int64) * N)[:, None, None])  # local [0,N)
    garange = np.arange(B)[:, None, None]

    in_maps = []
    for c in range(NCORES):
        jsl = slice(NC_ * c, NC_ * (c + 1))
        xs = xbf[:, jsl, :]                                # [128g, 128j, 32f]
        # xt[(q,f), j*32+grp] = x[grp*4+q, 128c+j, f]
        xt = np.ascontiguousarray(
            xs.reshape(GRP, 4, NC_, F).transpose(1, 3, 2, 0)
            .reshape(128, NC_ * GRP))
        # xe[(q,f), ((j*32)+grp)*16+d] = x[g, src_local[g, 128c+j, d], f]
        sl = sidx[:, jsl, :]                               # [128g, 128j, 16d]
        xg = xbf[garange, sl, :]                           # [g, j, d, f]
        xe = np.ascontiguousarray(
            xg.reshape(GRP, 4, NC_, DEG, F).transpose(1, 4, 2, 0, 3)
            .reshape(128, NC_ * GRP * DEG))
        # whead[h, j*76+o] = W_cat[o, 128c+j, h]
        whead = np.ascontiguousarray(
            W_cat[:, jsl, :].transpose(2, 1, 0).reshape(H, NC_ * NH)).astype(bf)
        in_maps.append({"xt": xt, "xe": xe, "whead": whead, **shared})
    return in_maps


def kernel(**inputs) -> np.ndarray:
    global LAST_RESULTS
    from concourse.bass_utils import run_bass_kernel_spmd

    if "nc" not in _CACHE:
        _CACHE["nc"] = _build_program()
    nc = _CACHE["nc"]
    in_maps = _make_in_maps(inputs)
    rr = run_bass_kernel_spmd(nc, in_maps, list(range(NCORES)))
    LAST_RESULTS = rr
    out = np.zeros((B, 3, 4), np.float32)
    for c in range(NCORES):
        out[c * BL:(c + 1) * BL] = rr.results[c]["out"].reshape(BL, 3, 4)
    return out


# revision 4
# speedup vs baseline: 1.5107x; 1.5107x over previous
"""Trainium2 Bass kernel for nn_BHS_SAGE (GNN message passing + dueling head).

Node-chunk sharding: core c owns nodes [128c, 128(c+1)) of ALL 128 graphs
(instead of 16 whole graphs).  The SAGE stages (pool-MLP, edge max-agg,
self+neigh matmul) see the same per-core work either way, but the dueling
head only needs this core's 128-node slice of W_adv/W_v1 (2.4 MB instead of
the full 19.9 MB replicated), and its matmuls run at M=128 (full PE rows).

Each core returns its per-graph head partial sums [128 g, 76] in fp32; the
host sums the 8 partials and applies the dueling tail (76 -> 12 outputs per
graph, ~60 KFLOP total vs ~9 GFLOP on device).  An on-device ReduceScatter
was measured at ~50 us of firmware latency for 39 KB and abandoned.

Per-core pipeline, 8 slabs (each slab = 16 dst nodes x 32 graph-groups):
  A. z = W_pool_blockdiag @ xe-slab (4-graph packed, 1024-col PSUM blocks)
  B. agg = relu(max_d z + b); two drain flavours balanced across engines:
       direct slab:   DVE reduce_max from PSUM (1x), bias+relu after (4x TS)
       assisted slab: ACT relu+bias drain PSUM->SBUF bf16, then a
                      slab-batched TT-max tree (2x_1p) on DVE
  D. h = relu(W_self x + W_neigh agg + b) per graph quadrant (ACT drain)
  E. head psum[128 g, 76] += ht[:, j].T @ whead[:, j]  (16 matmuls/slab,
     accumulated across all 128 j; hidden under the slab drains)
"""

import numpy as np

B, N, F, H, DEG = 128, 1024, 32, 128, 16
NCORES = 8
NC_ = N // NCORES         # 128 nodes per core chunk
BL = B // NCORES          # 16 output graphs per core (host tail bookkeeping)
GRP = B // 4              # 32 groups of 4 graphs packed per 128 partitions
NA = 12                   # adv outputs (3 branches x 4 actions)
NV = 64                   # val hidden
NH = NA + NV              # 76 combined head outputs
NSLAB = 8                 # j-slabs per core
JS = NC_ // NSLAB         # 16 dst nodes per slab
SLABC = JS * GRP * DEG    # 8192 xe cols per slab
BLK = 1024                # PSUM drain block (2 banks)
ASSIST = (1, 4, 6)        # slabs drained via ACT + TT-max tree

_CACHE = {}
LAST_RESULTS = None


def _build_program():
    import concourse.bass as bass
    import concourse.bacc as bacc
    import concourse.mybir as mybir
    import concourse.tile as tile

    f32 = mybir.dt.float32
    bf16 = mybir.dt.bfloat16
    Relu = mybir.ActivationFunctionType.Relu
    Alu = mybir.AluOpType

    nc = bacc.Bacc("TRN2", target_bir_lowering=False, debug=False,
                   num_devices=NCORES)

    # ---- kernel I/O ----
    xt_d = nc.declare_dram_parameter("xt", [128, NC_ * GRP], bf16, isOutput=False)
    xe_d = nc.declare_dram_parameter("xe", [128, NSLAB * SLABC], bf16, isOutput=False)
    wpool_d = nc.declare_dram_parameter("wpool_bd", [128, 128], bf16, isOutput=False)
    bpool_d = nc.declare_dram_parameter("bpool", [128, 1], f32, isOutput=False)
    wself_d = nc.declare_dram_parameter("wself_bd", [128, 4 * H], bf16, isOutput=False)
    wneigh_d = nc.declare_dram_parameter("wneigh_bd", [128, 4 * H], bf16, isOutput=False)
    bsage_d = nc.declare_dram_parameter("bsage", [128, 1], f32, isOutput=False)
    whead_d = nc.declare_dram_parameter("whead", [128, NC_ * NH], bf16, isOutput=False)
    hpart_d = nc.declare_dram_parameter("hpart", [128, NH], f32, isOutput=True)

    import os as _os
    _dbg = _os.environ.get("KDBG") == "1"
    if _dbg:
        dbg_aggT_d = nc.declare_dram_parameter("dbg_aggT", [128, NC_ * GRP], bf16, isOutput=True)
        dbg_ht_d = nc.declare_dram_parameter("dbg_ht", [128, NC_ * B], bf16, isOutput=True)

    with tile.TileContext(nc) as tc:
        with (
            tc.tile_pool(name="const", bufs=1) as cpool,
            tc.tile_pool(name="big", bufs=1) as bigpool,
        ):
            # ---- constants (ordered so slab-0 inputs land first) ----
            wpool = cpool.tile([128, 128], bf16)
            nc.sync.dma_start(out=wpool[:], in_=wpool_d[:])
            bpool = cpool.tile([128, 1], f32)
            nc.sync.dma_start(out=bpool[:], in_=bpool_d[:])
            xt = cpool.tile([128, NC_ * GRP], bf16)        # [(q,f), (j,grp)]
            nc.sync.dma_start(out=xt[:], in_=xt_d[:])
            wself = cpool.tile([128, 4 * H], bf16)
            nc.sync.dma_start(out=wself[:], in_=wself_d[:])
            wneigh = cpool.tile([128, 4 * H], bf16)
            nc.sync.dma_start(out=wneigh[:], in_=wneigh_d[:])
            bsage = cpool.tile([128, 1], f32)
            nc.sync.dma_start(out=bsage[:], in_=bsage_d[:])
            whead = cpool.tile([128, NC_ * NH], bf16)      # [h, (j, o)]

            ht = bigpool.tile([128, NC_ * B], bf16)        # [h, j*128 + g]  4MB
            aggT = bigpool.tile([128, NC_ * GRP], bf16)    # [(q,f'), j*32+grp] 1MB

            # head psum allocated up-front: accumulates across all slabs
            hd_ps_ctx = tc.tile_pool(name="hd_ps", bufs=1, space="PSUM")
            hd_ps = hd_ps_ctx.__enter__()
            pshead = hd_ps.tile([128, NH], f32)

            with (
                tc.tile_pool(name="xe_sb", bufs=3) as xe_pool,
                tc.tile_pool(name="z_ps", bufs=2, space="PSUM") as z_ps,
                tc.tile_pool(name="zr_sb", bufs=2) as zr_pool,
                tc.tile_pool(name="h_ps", bufs=2, space="PSUM") as h_ps,
            ):
                for s in range(NSLAB):
                    # ---- stage A+B: aggT slab = relu(max_d(W_pool@x[src]) + b) ----
                    # xe slab cols: (jj 16, grp 32, d 16); aggT cols: j*32+grp
                    xe = xe_pool.tile([128, SLABC], bf16, tag="xe")
                    nc.sync.dma_start(
                        out=xe[:], in_=xe_d[:, s * SLABC:(s + 1) * SLABC])
                    assisted = s in ASSIST
                    if assisted:
                        zr = zr_pool.tile([128, SLABC], bf16, tag="zr")
                    a0 = s * JS * GRP                      # aggT col offset
                    for blk in range(SLABC // BLK):   # 8 blocks of 1024 (64 nd, 16 d)
                        ps = z_ps.tile([128, BLK], f32, tag="zps")
                        for h2 in range(2):   # one matmul per PSUM bank
                            nc.tensor.matmul(
                                out=ps[:, h2 * 512:(h2 + 1) * 512],
                                lhsT=wpool[:],
                                rhs=xe[:, blk * BLK + h2 * 512:
                                        blk * BLK + (h2 + 1) * 512],
                                start=True, stop=True,
                            )
                        if assisted:
                            # fused relu+bias drain; max-tree comes after
                            nc.scalar.activation(
                                out=zr[:, blk * BLK:(blk + 1) * BLK],
                                in_=ps[:], func=Relu, bias=bpool[:])
                        else:
                            nc.vector.reduce_max(
                                out=aggT[:, a0 + blk * (BLK // DEG):
                                         a0 + (blk + 1) * (BLK // DEG)],
                                in_=ps[:].rearrange("p (n d) -> p n d", d=DEG),
                                axis=mybir.AxisListType.X)
                    if assisted:
                        # slab-batched TT-max tree over d (bf16, 2x_1p)
                        zrv = zr[:].rearrange("p (n d) -> p n d", d=DEG)
                        t1 = zr_pool.tile([128, JS * GRP * 8], bf16, tag="t1")
                        t1v = t1[:].rearrange("p (n d) -> p n d", d=8)
                        nc.vector.tensor_tensor(
                            out=t1v[:], in0=zrv[:, :, 0:8], in1=zrv[:, :, 8:16],
                            op=Alu.max)
                        t2 = zr_pool.tile([128, JS * GRP * 4], bf16, tag="t2")
                        t2v = t2[:].rearrange("p (n d) -> p n d", d=4)
                        nc.vector.tensor_tensor(
                            out=t2v[:], in0=t1v[:, :, 0:4], in1=t1v[:, :, 4:8],
                            op=Alu.max)
                        t3 = zr_pool.tile([128, JS * GRP * 2], bf16, tag="t3")
                        t3v = t3[:].rearrange("p (n d) -> p n d", d=2)
                        nc.vector.tensor_tensor(
                            out=t3v[:], in0=t2v[:, :, 0:2], in1=t2v[:, :, 2:4],
                            op=Alu.max)
                        nc.vector.tensor_tensor(
                            out=aggT[:, a0:a0 + JS * GRP],
                            in0=t3v[:, :, 0], in1=t3v[:, :, 1],
                            op=Alu.max)
                    else:
                        # relu+bias for the direct slab (single fast TS op)
                        nc.vector.tensor_scalar(
                            out=aggT[:, a0:a0 + JS * GRP],
                            in0=aggT[:, a0:a0 + JS * GRP],
                            scalar1=bpool[:], scalar2=0.0,
                            op0=Alu.add, op1=Alu.max)

                    # ---- stage D: ht slab = relu(W_self x + W_neigh agg + b) ----
                    # rhs cols (jj 16, grp 32) contiguous; out ht[j*128+grp*4+q]
                    htv = ht[:].rearrange("p (j grp q) -> p j grp q", grp=GRP, q=4)
                    for q in range(4):
                        hp = h_ps.tile([128, JS * GRP], f32, tag="hps")
                        nc.tensor.matmul(
                            out=hp[:],
                            lhsT=wself[:, q * H:(q + 1) * H],
                            rhs=xt[:, s * JS * GRP:(s + 1) * JS * GRP],
                            start=True, stop=False)
                        nc.tensor.matmul(
                            out=hp[:],
                            lhsT=wneigh[:, q * H:(q + 1) * H],
                            rhs=aggT[:, s * JS * GRP:(s + 1) * JS * GRP],
                            start=False, stop=True)
                        nc.scalar.activation(
                            out=htv[:, s * JS:(s + 1) * JS, :, q],
                            in_=hp[:].rearrange("p (j grp) -> p j grp", grp=GRP),
                            func=Relu, bias=bsage[:])

                    # ---- stage E: head accumulation over this slab's nodes ----
                    nc.sync.dma_start(
                        out=whead[:, s * JS * NH:(s + 1) * JS * NH],
                        in_=whead_d[:, s * JS * NH:(s + 1) * JS * NH])
                    for jj in range(JS):
                        j = s * JS + jj
                        nc.tensor.matmul(
                            out=pshead[:],
                            lhsT=ht[:, j * B:(j + 1) * B],
                            rhs=whead[:, j * NH:(j + 1) * NH],
                            start=(j == 0), stop=(j == NC_ - 1),
                        )

            if _dbg:
                nc.sync.dma_start(out=dbg_aggT_d[:], in_=aggT[:])
                nc.sync.dma_start(out=dbg_ht_d[:], in_=ht[:])

            # ---- output per-graph head partials; tail is summed on host ----
            with tc.tile_pool(name="tail", bufs=1) as tp:
                psf = tp.tile([128, NH], f32)
                nc.scalar.copy(out=psf[:], in_=pshead[:])
                nc.sync.dma_start(out=hpart_d[:], in_=psf[:])
            hd_ps_ctx.__exit__(None, None, None)
    nc.compile()
    return nc


def _make_in_maps(inputs):
    import ml_dtypes
    bf = ml_dtypes.bfloat16

    x = np.asarray(inputs["x"], np.float32)
    src = np.asarray(inputs["src"], np.int64)
    W_pool = np.asarray(inputs["W_pool"], np.float32)
    b_pool = np.asarray(inputs["b_pool"], np.float32)
    W_self = np.asarray(inputs["W_self"], np.float32)
    W_neigh = np.asarray(inputs["W_neigh"], np.float32)
    b_sage = np.asarray(inputs["b_sage"], np.float32)
    W_adv = np.asarray(inputs["W_adv"], np.float32)
    W_v1 = np.asarray(inputs["W_v1"], np.float32)

    # shared (replicated) tensors
    wpool_bd = np.kron(np.eye(4, dtype=np.float32), W_pool.T).astype(bf)  # [128,128]
    bpool = np.ascontiguousarray(np.tile(b_pool, 4)[:, None], np.float32)
    wself_bd = np.zeros((128, 4 * H), np.float32)
    wneigh_bd = np.zeros((128, 4 * H), np.float32)
    for q in range(4):
        wself_bd[q * 32:(q + 1) * 32, q * H:(q + 1) * H] = W_self.T
        wneigh_bd[q * 32:(q + 1) * 32, q * H:(q + 1) * H] = W_neigh.T
    bsage = np.ascontiguousarray(b_sage[:, None])
    W_cat = np.concatenate([W_adv, W_v1], axis=0)          # [76, 131072]
    W_cat = W_cat.reshape(NH, N, H)                        # [o, n, h]

    shared = {
        "wpool_bd": wpool_bd, "bpool": bpool,
        "wself_bd": wself_bd.astype(bf), "wneigh_bd": wneigh_bd.astype(bf),
        "bsage": bsage,
    }

    xbf = x.astype(bf)                                     # [128, 1024, 32]
    sidx = (src.reshape(B, N, DEG)
            - (np.arange(B, dtype=np.int64) * N)[:, None, None])  # local [0,N)
    garange = np.arange(B)[:, None, None]

    in_maps = []
    for c in range(NCORES):
        jsl = slice(NC_ * c, NC_ * (c + 1))
        xs = xbf[:, jsl, :]                                # [128g, 128j, 32f]
        # xt[(q,f), j*32+grp] = x[grp*4+q, 128c+j, f]
        xt = np.ascontiguousarray(
            xs.reshape(GRP, 4, NC_, F).transpose(1, 3, 2, 0)
            .reshape(128, NC_ * GRP))
        # xe[(q,f), ((j*32)+grp)*16+d] = x[g, src_local[g, 128c+j, d], f]
        sl = sidx[:, jsl, :]                               # [128g, 128j, 16d]
        xg = xbf[garange, sl, :]                           # [g, j, d, f]
        xe = np.ascontiguousarray(
            xg.reshape(GRP, 4, NC_, DEG, F).transpose(1, 4, 2, 0, 3)
            .reshape(128, NC_ * GRP * DEG))
        # whead[h, j*76+o] = W_cat[o, 128c+j, h]
        whead = np.ascontiguousarray(
            W_cat[:, jsl, :].transpose(2, 1, 0).reshape(H, NC_ * NH)).astype(bf)
        in_maps.append({"xt": xt, "xe": xe, "whead": whead, **shared})
    return in_maps


def _host_tail(hsum, inputs):
    """Dueling tail on the summed head partials [128, 76] (fp32, tiny)."""
    b_adv = np.asarray(inputs["b_adv"], np.float32)
    b_v1 = np.asarray(inputs["b_v1"], np.float32)
    W_v2 = np.asarray(inputs["W_v2"], np.float32)
    b_v2 = np.asarray(inputs["b_v2"], np.float32)
    W_v3 = np.asarray(inputs["W_v3"], np.float32)
    b_v3 = np.asarray(inputs["b_v3"], np.float32)

    adv = np.maximum(hsum[:, :NA] + b_adv, 0.0).reshape(B, 3, 4)
    val = np.maximum(hsum[:, NA:] + b_v1, 0.0)
    val = np.maximum(val @ W_v2.T + b_v2, 0.0)
    val = val @ W_v3.T + b_v3                               # [B, 1]
    return val[..., None] + adv - adv.mean(-1, keepdims=True)


def kernel(**inputs) -> np.ndarray:
    global LAST_RESULTS
    from concourse.bass_utils import run_bass_kernel_spmd

    if "nc" not in _CACHE:
        _CACHE["nc"] = _build_program()
    nc = _CACHE["nc"]
    in_maps = _make_in_maps(inputs)
    rr = run_bass_kernel_spmd(nc, in_maps, list(range(NCORES)))
    LAST_RESULTS = rr
    hsum = np.zeros((B, NH), np.float32)
    for c in range(NCORES):
        hsum += rr.results[c]["hpart"]
    return _host_tail(hsum, inputs).astype(np.float32)
